# revision 14
# baseline (speedup 1.0000x reference)
"""BiMultiHeadAttention Trainium2 kernel.

Sharding: batch x query-half across 8 cores. Core c handles batch b=c>>1 and
query rows R = [1024*j, 1024*(j+1)), j=c&1, for ALL 8 heads.

Per-core pipeline:
  - projections (PE, f32r): QT[d, q_local], KT[d, k] transposed; V/Vq fp16 natural
  - S = Q K^T per head via row-tiled K=32 matmuls (f32r) -> PSUM
  - P = exp(S/sqrt(D)) on ScalarE (fp16 out, fused row-sum via accum_out)
  - P *= 1/rowsum on GpSimd (per-partition scalar)
  - PT via xbar DMA transpose (3/4) + PE transpose-mode (1/4)
  - out_r^T[d, k] = sum_q Vq[q,d] P[q,k]  (PE fp16, col-tiled heads)
  - out_l^T[d, q] = sum_k V[k,d] PT[k,q]  (PE fp16, col-tiled heads)
  - out projections with Wp (f32r)
  - out_l fully local -> LN + residual -> out_l
  - out_r partial over local q: 8-rank AllGather exchange, partner/self slots
    read back via host-passed dynamic offsets; sum -> +bp -> LN + residual
"""

import numpy as np

B, N, E, H, D = 4, 2048, 256, 8, 32
EPS = 1e-5
NQ = N // 2
NCORES = 8
SCALE = float(1.0 / np.sqrt(D))

_CACHE = {}


def _build_program():
    import sys
    if "/opt/trn_rl_repo" not in sys.path:
        sys.path.insert(0, "/opt/trn_rl_repo")
    from contextlib import ExitStack
    import concourse.bass as bass
    import concourse.tile as tile
    from concourse import bacc
    from concourse import mybir
    from concourse.masks import make_identity

    F32 = mybir.dt.float32
    F32R = mybir.dt.float32r
    F16 = mybir.dt.float16
    U32 = mybir.dt.uint32
    EXP = mybir.ActivationFunctionType.Exp
    SQRT = mybir.ActivationFunctionType.Sqrt
    ADDOP = mybir.AluOpType.add
    SUBOP = mybir.AluOpType.subtract
    MULOP = mybir.AluOpType.mult

    nc = bacc.Bacc(num_devices=NCORES)

    inp = {}

    def P_(name, shape, dt=F32):
        inp[name] = nc.declare_dram_parameter(name, list(shape), dt, isOutput=False)

    P_("xlT", (E, N), F32R); P_("xrT", (E, N), F32R)
    P_("xlTq", (E, NQ), F32R); P_("xrTq", (E, NQ), F32R)
    P_("w5h", (E, 5, E), F32R)        # (e, {q,k,v,vn,p}, d)
    P_("bqk2", (E, 2))                # (d, {bq,bk})
    P_("rows6", (6, E))               # {bv,bp,gl,bl,gr,br}
    P_("xresl", (NQ, E)); P_("xresr", (NQ, E))
    P_("offs", (1, 2), U32)           # {self_off, partner_off}

    out_l_d = nc.declare_dram_parameter("out_l", [NQ, E], F32, isOutput=True)
    out_r_d = nc.declare_dram_parameter("out_r", [NQ, E], F32, isOutput=True)

    xchg_send = nc.dram_tensor("xchg_send", [N, E], F32)
    xchg_out = nc.dram_tensor("xchg_out", [NCORES * N, E], F32, addr_space="Shared")

    with tile.TileContext(nc) as tc:
        with ExitStack() as ctx:
            const = ctx.enter_context(tc.tile_pool(name="const", bufs=1))
            aux = ctx.enter_context(tc.tile_pool(name="aux", bufs=1))
            xin = ctx.enter_context(tc.tile_pool(name="xin", bufs=2))
            stand = ctx.enter_context(tc.tile_pool(name="stand", bufs=1))
            ppool = ctx.enter_context(tc.tile_pool(name="ppool", bufs=12))
            dpool = ctx.enter_context(tc.tile_pool(name="dpool", bufs=2))
            lt3 = ctx.enter_context(tc.tile_pool(name="lt3", bufs=2))

            s_ps = ctx.enter_context(tc.tile_pool(name="s_ps", bufs=2, space="PSUM"))
            r_ps = ctx.enter_context(tc.tile_pool(name="r_ps", bufs=2, space="PSUM"))
            m_ps = ctx.enter_context(tc.tile_pool(name="m_ps", bufs=2, space="PSUM"))

            # ------------- constants -------------
            w5 = const.tile([128, 2, 5, E], F32R, name="w5")  # (ech, {q,k,v,vn,p}, d)
            nc.sync.dma_start(
                out=w5, in_=inp["w5h"].rearrange("(c p) f d -> p c f d", p=128))
            bias2d = const.tile([128, 2, 2], F32, name="bias2d")  # (dh, {bq,bk})
            nc.sync.dma_start(
                out=bias2d, in_=inp["bqk2"].rearrange("(c p) k -> p c k", p=128))
            bias2 = const.tile([128, 2, 2], F32, name="bias2")
            nc.vector.tensor_copy(out=bias2, in_=bias2d)
            rows_d = const.tile([128, 6, E], F32, name="rows_d")
            r6 = inp["rows6"]
            nc.sync.dma_start(
                out=rows_d,
                in_=bass.AP(tensor=r6[:, :].tensor, offset=0,
                            ap=[[0, 128], [E, 6], [1, E]]))
            rows = const.tile([128, 6, E], F32, name="rows")
            nc.vector.tensor_copy(out=rows, in_=rows_d)
            bv_b = rows[:, 0, :]
            ident = const.tile([128, 128], F16, name="ident")
            make_identity(nc, ident)
            eps_t = aux.tile([128, 1], F32, name="eps_t")
            nc.vector.memset(eps_t, EPS)

            # ------------- projections -------------
            QT = stand.tile([128, 2, NQ], F32R, name="QT")
            KT = stand.tile([128, 2, N], F32R, name="KT")
            V = stand.tile([128, 16, E], F16, name="V")
            Vq = stand.tile([128, 8, E], F16, name="Vq")

            CH = 256  # free-dim chunk of the streamed inputs
            for nck in range(N // CH):
                sl = slice(nck * CH, (nck + 1) * CH)
                xl_c = xin.tile([128, 2, CH], F32R, tag="xl")
                xr_c = xin.tile([128, 2, CH], F32R, tag="xr")
                nc.sync.dma_start(
                    out=xl_c, in_=inp["xlT"].rearrange("(c p) n -> p c n", p=128)[:, :, sl])
                nc.sync.dma_start(
                    out=xr_c, in_=inp["xrT"].rearrange("(c p) n -> p c n", p=128)[:, :, sl])
                # KT (transposed layout)
                for dh in range(2):
                    kps = m_ps.tile([128, CH], F32, tag="m")
                    for ech in range(2):
                        nc.tensor.matmul(
                            kps, lhsT=w5[:, ech, 1, dh * 128:(dh + 1) * 128],
                            rhs=xr_c[:, ech, :],
                            start=(ech == 0), stop=(ech == 1))
                    nc.vector.tensor_scalar(
                        out=KT[:, dh, sl], in0=kps,
                        scalar1=bias2[:, dh, 1:2], scalar2=None, op0=ADDOP)
                # V natural layout, fp16
                for i in range(CH // 128):
                    nt = nck * (CH // 128) + i
                    vps = m_ps.tile([128, E], F32, tag="m")
                    for ech in range(2):
                        nc.tensor.matmul(
                            vps, lhsT=xl_c[:, ech, i * 128:(i + 1) * 128],
                            rhs=w5[:, ech, 2, :],
                            start=(ech == 0), stop=False)
                    for ech in range(2):
                        nc.tensor.matmul(
                            vps, lhsT=xr_c[:, ech, i * 128:(i + 1) * 128],
                            rhs=w5[:, ech, 3, :],
                            start=False, stop=(ech == 1))
                    nc.vector.tensor_tensor(
                        out=V[:, nt, :], in0=vps, in1=bv_b, op=ADDOP)

            for nck in range(NQ // CH):
                sl = slice(nck * CH, (nck + 1) * CH)
                xl_c = xin.tile([128, 2, CH], F32R, tag="xl")
                xr_c = xin.tile([128, 2, CH], F32R, tag="xr")
                nc.sync.dma_start(
                    out=xl_c, in_=inp["xlTq"].rearrange("(c p) n -> p c n", p=128)[:, :, sl])
                nc.sync.dma_start(
                    out=xr_c, in_=inp["xrTq"].rearrange("(c p) n -> p c n", p=128)[:, :, sl])
                for dh in range(2):
                    qps = m_ps.tile([128, CH], F32, tag="m")
                    for ech in range(2):
                        nc.tensor.matmul(
                            qps, lhsT=w5[:, ech, 0, dh * 128:(dh + 1) * 128],
                            rhs=xl_c[:, ech, :],
                            start=(ech == 0), stop=(ech == 1))
                    nc.vector.tensor_scalar(
                        out=QT[:, dh, sl], in0=qps,
                        scalar1=bias2[:, dh, 0:1], scalar2=None, op0=ADDOP)
                for i in range(CH // 128):
                    nt = nck * (CH // 128) + i
                    vps = m_ps.tile([128, E], F32, tag="m")
                    for ech in range(2):
                        nc.tensor.matmul(
                            vps, lhsT=xl_c[:, ech, i * 128:(i + 1) * 128],
                            rhs=w5[:, ech, 2, :],
                            start=(ech == 0), stop=False)
                    for ech in range(2):
                        nc.tensor.matmul(
                            vps, lhsT=xr_c[:, ech, i * 128:(i + 1) * 128],
                            rhs=w5[:, ech, 3, :],
                            start=False, stop=(ech == 1))
                    nc.vector.tensor_tensor(
                        out=Vq[:, nt, :], in0=vps, in1=bv_b, op=ADDOP)

            # ------------- standing accumulators -------------
            outlT = stand.tile([128, 2, NQ], F32R, name="outlT")
            outrT = stand.tile([128, 2, N], F32R, name="outrT")
            PT = stand.tile([128, 8, 16, 256], F16, name="PT")  # (h, kt, q_sb)

            # ------------- main loop: 4 superblocks x 2 q-chunks x 8 heads ----
            for sb in range(4):
                p_tiles = {}
                for qc in range(2):
                    qg = sb * 2 + qc
                    dacc = dpool.tile([128, 8, 2], F32, tag="dacc")
                    for h in range(8):
                        th, hh = h // 4, h % 4
                        pt = ppool.tile([128, N], F16, tag="p")
                        p_tiles[(h, qc)] = pt
                        for kh in range(2):
                            st = s_ps.tile([128, 1024], F32, tag="s")
                            for i in range(2):
                                nc.tensor.matmul(
                                    st[:, i * 512:(i + 1) * 512],
                                    lhsT=QT[32 * hh:32 * hh + 32, th, qg * 128:(qg + 1) * 128],
                                    rhs=KT[32 * hh:32 * hh + 32, th,
                                           kh * 1024 + i * 512:kh * 1024 + (i + 1) * 512],
                                    start=True, stop=True,
                                    tile_position=(32 * hh, 0))
                            nc.scalar.activation(
                                out=pt[:, kh * 1024:(kh + 1) * 1024], in_=st,
                                func=EXP, scale=SCALE,
                                accum_out=dacc[:, h, kh:kh + 1])
                    dinv = dpool.tile([128, 8], F32, tag="dinv")
                    nc.vector.tensor_tensor(
                        out=dinv, in0=dacc[:, :, 0], in1=dacc[:, :, 1], op=ADDOP)
                    nc.vector.reciprocal(out=dinv, in_=dinv)
                    for h in range(8):
                        pt = p_tiles[(h, qc)]
                        nc.gpsimd.tensor_scalar_mul(
                            out=pt, in0=pt, scalar1=dinv[:, h:h + 1])
                        if qc == 0 and h % 2 == 0:  # PE transpose share: 4/16
                            for kt0 in (0, 8):
                                tp = m_ps.tile([128, 8, 128], F16, tag="m")
                                for i in range(8):
                                    nc.tensor.transpose(
                                        tp[:, i, :],
                                        pt[:, (kt0 + i) * 128:(kt0 + i + 1) * 128],
                                        ident)
                                nc.vector.tensor_copy(
                                    out=PT[:, h, kt0:kt0 + 8, qc * 128:(qc + 1) * 128],
                                    in_=tp)
                        else:
                            nc.scalar.dma_start_transpose(
                                out=PT[:, h, :, qc * 128:(qc + 1) * 128], in_=pt)

                    # out_r^T partial for this q-chunk
                    for dh in range(2):
                        for kc in range(4):
                            rp = r_ps.tile([128, 512], F32, tag="r")
                            for hh in range(4):
                                nc.tensor.matmul(
                                    rp[32 * hh:32 * hh + 32, :],
                                    lhsT=Vq[:, qg, dh * 128 + 32 * hh:dh * 128 + 32 * hh + 32],
                                    rhs=p_tiles[(4 * dh + hh, qc)][:, kc * 512:(kc + 1) * 512],
                                    start=True, stop=True,
                                    tile_position=(0, 32 * hh),
                                    skip_group_check=True)
                            if sb == 0 and qc == 0:
                                nc.vector.tensor_copy(
                                    out=outrT[:, dh, kc * 512:(kc + 1) * 512], in_=rp)
                            else:
                                nc.vector.tensor_tensor(
                                    out=outrT[:, dh, kc * 512:(kc + 1) * 512],
                                    in0=outrT[:, dh, kc * 512:(kc + 1) * 512],
                                    in1=rp, op=ADDOP)

                # out_l^T[:, sb]: contraction over all k via PT
                for dh in range(2):
                    lp = m_ps.tile([128, 256], F32, tag="m")
                    for kt in range(16):
                        for hh in range(4):
                            nc.tensor.matmul(
                                lp[32 * hh:32 * hh + 32, :],
                                lhsT=V[:, kt, dh * 128 + 32 * hh:dh * 128 + 32 * hh + 32],
                                rhs=PT[:, 4 * dh + hh, kt, :],
                                start=(kt == 0), stop=(kt == 15),
                                tile_position=(0, 32 * hh),
                                skip_group_check=True)
                    nc.vector.tensor_copy(
                        out=outlT[:, dh, sb * 256:(sb + 1) * 256], in_=lp)

            # ------------- output projections -------------
            bp_b = rows[:, 1, :]

            projl = [ppool.tile([128, 4, E], F32, tag="p", name=f"projl{i}") for i in range(2)]
            for qt in range(8):
                pp = m_ps.tile([128, 256], F32, tag="m")
                for dh in range(2):
                    nc.tensor.matmul(
                        pp, lhsT=outlT[:, dh, qt * 128:(qt + 1) * 128],
                        rhs=w5[:, dh, 4, :],
                        start=(dh == 0), stop=(dh == 1))
                nc.vector.tensor_tensor(
                    out=projl[qt // 4][:, qt % 4, :], in0=pp, in1=bp_b, op=ADDOP)

            projr = [ppool.tile([128, 4, E], F32, tag="p", name=f"projr{i}") for i in range(4)]
            for ntk in range(16):
                pp = m_ps.tile([128, 256], F32, tag="m")
                for dh in range(2):
                    nc.tensor.matmul(
                        pp, lhsT=outrT[:, dh, ntk * 128:(ntk + 1) * 128],
                        rhs=w5[:, dh, 4, :],
                        start=(dh == 0), stop=(dh == 1))
                nc.vector.tensor_copy(out=projr[ntk // 4][:, ntk % 4, :], in_=pp)

            for ch in range(4):
                nc.sync.dma_start(
                    out=xchg_send.rearrange("(c t p) e -> c p t e", c=4, p=128)[ch],
                    in_=projr[ch])

            nc.gpsimd.collective_compute(
                "AllGather", mybir.AluOpType.bypass,
                replica_groups=[list(range(NCORES))],
                ins=[xchg_send[:, :]],
                outs=[xchg_out[:, :]],
            )

            # ------------- LN + residual -------------
            gl_b = rows[:, 2, :]
            bl_b = rows[:, 3, :]
            gr_b = rows[:, 4, :]
            br_b = rows[:, 5, :]

            def layernorm_apply(z_chunks, gamma_b, beta_b, xres_dram, out_dram):
                nch = len(z_chunks)
                xres = [ppool.tile([128, 4, E], F32, tag="p", name=f"xres{i}") for i in range(nch)]
                for ch in range(nch):
                    nc.sync.dma_start(
                        out=xres[ch],
                        in_=xres_dram.rearrange("(c t p) e -> c p t e", c=nch, p=128)[ch])
                    nc.vector.tensor_tensor(
                        out=xres[ch], in0=xres[ch],
                        in1=beta_b.unsqueeze(1).broadcast_to((128, 4, E)), op=ADDOP)
                stats = lt3.tile([128, 8, 6], F32, tag="lns")
                mv = lt3.tile([128, 8, 2], F32, tag="lnm")
                for t in range(8):
                    nc.vector.bn_stats(out=stats[:, t, :], in_=z_chunks[t // 4][:, t % 4, :])
                    nc.vector.bn_aggr(out=mv[:, t, :], in_=stats[:, t, :])
                nc.scalar.activation(
                    out=mv[:, :, 1], in_=mv[:, :, 1], func=SQRT, bias=eps_t)
                nc.vector.reciprocal(out=mv[:, :, 1], in_=mv[:, :, 1])
                for t in range(8):
                    z = z_chunks[t // 4][:, t % 4, :]
                    nc.vector.tensor_scalar(
                        out=z, in0=z,
                        scalar1=mv[:, t, 0:1], scalar2=mv[:, t, 1:2],
                        op0=SUBOP, op1=MULOP)
                for ch in range(nch):
                    nc.vector.tensor_tensor(
                        out=z_chunks[ch], in0=z_chunks[ch],
                        in1=gamma_b.unsqueeze(1).broadcast_to((128, 4, E)), op=MULOP)
                    nc.vector.tensor_tensor(
                        out=z_chunks[ch], in0=z_chunks[ch], in1=xres[ch], op=ADDOP)
                    nc.sync.dma_start(
                        out=out_dram.rearrange("(c t p) e -> c p t e", c=nch, p=128)[ch],
                        in_=z_chunks[ch])

            layernorm_apply(projl, gl_b, bl_b, inp["xresl"], out_l_d)

            # out_r: read self+partner slots from gathered buffer
            offt = aux.tile([1, 2], U32, name="offt")
            nc.sync.dma_start(out=offt, in_=inp["offs"][:, :])
            with tc.tile_critical():
                reg_s = nc.sync.alloc_register("self_off_r")
                nc.sync.reg_load(reg_s, offt[0:1, 0:1])
                sv_s = nc.sync.snap(reg_s, donate=True, min_val=0, max_val=NCORES * N - NQ)
                reg_p = nc.sync.alloc_register("partner_off_r")
                nc.sync.reg_load(reg_p, offt[0:1, 1:2])
                sv_p = nc.sync.snap(reg_p, donate=True, min_val=0, max_val=NCORES * N - NQ)
            zr = []
            for ch in range(2):
                rs = ppool.tile([128, 4, E], F32, tag="p", name=f"rs{ch}")
                rp_ = ppool.tile([128, 4, E], F32, tag="p", name=f"rp{ch}")
                nc.sync.dma_start(
                    out=rs,
                    in_=xchg_out[bass.ds(sv_s, NQ), :].rearrange(
                        "(c t p) e -> c p t e", c=2, p=128)[ch])
                nc.sync.dma_start(
                    out=rp_,
                    in_=xchg_out[bass.ds(sv_p, NQ), :].rearrange(
                        "(c t p) e -> c p t e", c=2, p=128)[ch])
                nc.vector.tensor_tensor(out=rs, in0=rs, in1=rp_, op=ADDOP)
                nc.vector.tensor_tensor(
                    out=rs, in0=rs,
                    in1=bp_b.unsqueeze(1).broadcast_to((128, 4, E)), op=ADDOP)
                zr.append(rs)
            layernorm_apply(zr, gr_b, br_b, inp["xresr"], out_r_d)

    nc.finalize()
    return nc


def _shard_inputs(x_l, x_r, Wq, bq, Wk, bk, Wv, bv, Wp, bp, ln_g, ln_b, rn_g, rn_b):
    f32 = np.float32
    in_maps = []
    for c in range(NCORES):
        b, j = c >> 1, c & 1
        R = slice(NQ * j, NQ * (j + 1))
        xlT = np.ascontiguousarray(np.asarray(x_l)[b].T, dtype=f32)
        xrT = np.ascontiguousarray(np.asarray(x_r)[b].T, dtype=f32)
        w5h = np.stack([np.asarray(Wq), np.asarray(Wk), np.asarray(Wv),
                        -np.asarray(Wv), np.asarray(Wp)], axis=1).astype(f32)
        rows6 = np.stack([np.asarray(bv), np.asarray(bp), np.asarray(ln_g),
                          np.asarray(ln_b), np.asarray(rn_g),
                          np.asarray(rn_b)]).astype(f32)
        m = {
            "xlT": xlT, "xrT": xrT,
            "xlTq": np.ascontiguousarray(xlT[:, R]),
            "xrTq": np.ascontiguousarray(xrT[:, R]),
            "w5h": np.ascontiguousarray(w5h),
            "bqk2": np.ascontiguousarray(
                np.stack([np.asarray(bq), np.asarray(bk)], axis=1).astype(f32)),
            "rows6": np.ascontiguousarray(rows6),
            "xresl": np.ascontiguousarray(np.asarray(x_l)[b][R], f32),
            "xresr": np.ascontiguousarray(np.asarray(x_r)[b][R], f32),
            "offs": np.array([[c * N + j * NQ, (c ^ 1) * N + j * NQ]],
                             dtype=np.uint32),
        }
        in_maps.append(m)
    return in_maps


def kernel(**inputs):
    import sys
    if "/opt/trn_rl_repo" not in sys.path:
        sys.path.insert(0, "/opt/trn_rl_repo")
    from concourse.bass_utils import run_bass_kernel_spmd

    if "nc" not in _CACHE:
        _CACHE["nc"] = _build_program()
    nc = _CACHE["nc"]

    in_maps = _shard_inputs(**{k: np.asarray(v) for k, v in inputs.items()})
    res = run_bass_kernel_spmd(nc, in_maps, core_ids=list(range(NCORES)))
    _CACHE["last_res"] = res
    out_l = np.empty((B, N, E), np.float32)
    out_r = np.empty((B, N, E), np.float32)
    for c in range(NCORES):
        b, j = c >> 1, c & 1
        R = slice(NQ * j, NQ * (j + 1))
        out_l[b, R] = res.results[c]["out_l"]
        out_r[b, R] = res.results[c]["out_r"]
    return out_l, out_r
